# revision 11
# baseline (speedup 1.0000x reference)
"""Longformer attention Bass kernel for 8 TRN2 NeuronCores.

Sharding: core c handles batch b = c//4 and heads 4*(c%4) .. 4*(c%4)+3.
Each core computes its 4 heads' attention + the partial output projection;
the host sums the 4 partials per batch element and adds the folded bias.

All matmuls run as float32r (full-rate on TRN2 when N >= 256).
Scores are computed transposed (s^T [keys, q]) so softmax needs no
on-chip transposes: denom comes from a ones-column appended to V, and the
per-query reciprocal is broadcast across partitions with a rank-1 matmul.
"""

import os
import numpy as np

import concourse.bass as bass
import concourse.mybir as mybir
import concourse.tile as tile
from concourse import bacc
from concourse.bass_utils import run_bass_kernel_spmd

# ---- problem constants (hardcoded per contract) ----
B, S, DM = 2, 2048, 1024
H, DH = 16, 64
WINDOW = 128
NG = max(1, int(S * 0.1))  # 204 global tokens
SCALE = 1.0 / np.sqrt(DH)
NCORES = 8
HPC = 4            # heads per core
F = HPC * DH       # 256 per-core head features
KB = S // 128      # 16 key blocks
NPAIR = 4          # ctx psum pair tiles (512 q cols each)

FP = mybir.dt.float32
FPR = mybir.dt.float32r
AF = mybir.ActivationFunctionType


# ---------------------------------------------------------------- planning
def _allow():
    pos = np.arange(S)
    dist = pos[None, :] - pos[:, None]
    window = np.abs(dist) <= WINDOW // 2
    isg = pos < NG
    return window | isg[:, None] | isg[None, :]  # [query i, key j]


def _keyset(g):
    if g == 0:
        return list(range(KB))
    s = {0, 1}
    for c in range(2 * g - 1, 2 * g + 3):
        if 0 <= c < KB:
            s.add(c)
    return sorted(s)


def _plan():
    """Static schedule: list of score/PV streams + deduped mask tiles.

    Each stream: dict(c, q0, w, pair, off, start, stop, ops)
      ops: ('mul', mask_idx, off)            - 256-wide mask multiply
           ('memset', p0, p1, c0, c1)        - zero rows p0:p1 x cols c0:c1
    """
    allowT = _allow().T  # [key, query]
    ksets = [_keyset(g) for g in range(8)]
    last_c = [max(ks) for ks in ksets]
    masks = []
    midx = {}
    streams = []

    def mask_id(sub):
        key = sub.tobytes()
        if key not in midx:
            midx[key] = len(masks)
            masks.append(sub.astype(np.float32))
        return midx[key]

    for c in range(KB):
        if c < 2:
            specs = [(512 * p, 512, p, 0) for p in range(4)]
        else:
            specs = [(256 * g, 256, g // 2, (g % 2) * 256)
                     for g in range(8) if c in ksets[g]]
        for q0, w, pair, off in specs:
            tileT = allowT[c * 128:(c + 1) * 128, q0:q0 + w]
            ops = []
            for o in range(0, w, 256):
                sub = tileT[:, o:o + 256]
                if sub.all():
                    continue
                rfull = sub.all(axis=1)
                rnone = ~sub.any(axis=1)
                cfull = sub.all(axis=0)
                cnone = ~sub.any(axis=0)
                if (rfull | rnone).all() and rnone.any():
                    # rows are all-true or all-false: memset the false rows
                    # (engines require 32-aligned partition offsets)
                    (idx,) = np.nonzero(rnone)
                    p0, p1 = idx.min(), idx.max() + 1
                    if (rnone[p0:p1].all() and p1 - p0 == len(idx)
                            and p0 % 32 == 0 and p1 % 32 == 0):
                        ops.append(("memset", int(p0), int(p1), o, o + 256))
                        continue
                if (cfull | cnone).all() and cnone.any():
                    (idx,) = np.nonzero(cnone)
                    c0, c1 = idx.min(), idx.max() + 1
                    if cnone[c0:c1].all() and c1 - c0 == len(idx):
                        ops.append(("memset", 0, 128, o + int(c0), o + int(c1)))
                        continue
                ops.append(("mul", mask_id(sub), o))
            if w == 512:
                start = (c == 0)
                stop = False  # window streams always finish each half later
            else:
                g = q0 // 256
                start = False
                stop = (c == last_c[g])
            streams.append(dict(c=c, q0=q0, w=w, pair=pair, off=off,
                                start=start, stop=stop, ops=ops))
    return streams, np.stack(masks)  # masks [nm, 128, 256]


# ---------------------------------------------------------------- builder
def _emit(tc, streams, nm, aps):
    nc = tc.nc
    hT, wqT, wkT, wvT, woT, bq2, bk2, mks, out = aps

    with tc.tile_pool(name="const", bufs=1) as const:
        hT_sb = const.tile([128, 8, S], FPR, tag="hT")
        wq_sb = const.tile([128, 8, F], FPR, tag="wq")
        wk_sb = const.tile([128, 8, F], FPR, tag="wk")
        wv_sb = const.tile([128, 8, F], FPR, tag="wv")
        wo_sb = const.tile([128, 2, DM], FPR, tag="wo")
        bq_sb = const.tile([128, 2], FP, tag="bq")
        bk_sb = const.tile([128, 2], FP, tag="bk")
        mk_sb = const.tile([128, nm, 256], FPR, tag="mk")
        ones_sb = const.tile([1, DH], FPR, tag="ones")
        qT_sb = const.tile([128, 2, S], FPR, tag="qT")
        kT_sb = const.tile([128, 2, S], FPR, tag="kT")
        v_sb = const.tile([128, HPC, KB, DH + 1], FPR, tag="v")
        cT_sb = const.tile([128, 2, S], FPR, tag="cT")

        hT_r = hT.rearrange("(t p) n -> p t n", p=128)
        for kt in range(8):
            nc.sync.dma_start(hT_sb[:, kt, :], hT_r[:, kt, :])
        nc.sync.dma_start(wq_sb[:], wqT.rearrange("(t p) f -> p t f", p=128))
        nc.sync.dma_start(wk_sb[:], wkT.rearrange("(t p) f -> p t f", p=128))
        nc.sync.dma_start(wv_sb[:], wvT.rearrange("(t p) f -> p t f", p=128))
        nc.sync.dma_start(wo_sb[:], woT.rearrange("(t p) f -> p t f", p=128))
        nc.sync.dma_start(bq_sb[:], bq2.rearrange("t p -> p t"))
        nc.sync.dma_start(bk_sb[:], bk2.rearrange("t p -> p t"))
        nc.sync.dma_start(mk_sb[:], mks.rearrange("n p m -> p n m"))
        # Memset can't codegen for f32r APs; the bit pattern of 1.0f is the
        # same, so write through a float32-bitcast view
        nc.vector.memset(ones_sb[:].bitcast(FP), 1.0)
        nc.vector.memset(v_sb[:, :, :, DH:DH + 1].bitcast(FP), 1.0)

        # ---- phase 1: projections q^T, k^T (feat-major) and v (seq-major)
        with tc.tile_pool(name="psproj", bufs=2, space="PSUM") as pp:
            for wsb, bsb, dst in ((wq_sb, bq_sb, qT_sb), (wk_sb, bk_sb, kT_sb)):
                for mt in range(2):
                    for nt in range(4):
                        ps = pp.tile([128, 512], FP, tag="pqk")
                        for kt in range(8):
                            nc.tensor.matmul(
                                ps[:], wsb[:, kt, mt * 128:(mt + 1) * 128],
                                hT_sb[:, kt, nt * 512:(nt + 1) * 512],
                                start=(kt == 0), stop=(kt == 7))
                        nc.vector.tensor_scalar_add(
                            dst[:, mt, nt * 512:(nt + 1) * 512], ps[:],
                            bsb[:, mt:mt + 1])
            for st in range(KB):
                ps = pp.tile([128, F], FP, tag="pv")
                for kt in range(8):
                    nc.tensor.matmul(
                        ps[:], hT_sb[:, kt, st * 128:(st + 1) * 128],
                        wv_sb[:, kt, :], start=(kt == 0), stop=(kt == 7))
                nc.scalar.activation(
                    v_sb[:, :, st, 0:DH],
                    ps.rearrange("p (h d) -> p h d", h=HPC), AF.Copy)

        # ---- phase 2: attention per head
        with tc.tile_pool(name="psT", bufs=2, space="PSUM") as psTp, \
             tc.tile_pool(name="pctx", bufs=1, space="PSUM") as pctx, \
             tc.tile_pool(name="pbc", bufs=2, space="PSUM") as pbc, \
             tc.tile_pool(name="work", bufs=8) as work, \
             tc.tile_pool(name="rcp", bufs=2) as rcp:
            for h in range(HPC):
                p0, mt = 64 * (h % 2), h // 2
                kT_h = kT_sb[p0:p0 + 64, mt, :]
                qT_h = qT_sb[p0:p0 + 64, mt, :]
                ctx = [pctx.tile([DH + 1, 512], FP, tag=f"ctx{p}", bufs=1,
                                 name=f"ctx{p}_{h}")
                       for p in range(NPAIR)]
                for st in streams:
                    c, q0, w = st["c"], st["q0"], st["w"]
                    ps = psTp.tile([128, 512], FP, tag="sT", name=f"sT{h}_{c}_{q0}")
                    nc.tensor.matmul(
                        ps[:, 0:w], kT_h[:, c * 128:(c + 1) * 128],
                        qT_h[:, q0:q0 + w], start=True, stop=True)
                    ex = work.tile([128, 512], FPR, tag="ex", name=f"ex{h}_{c}_{q0}")
                    nc.scalar.activation(ex[:, 0:w], ps[:, 0:w], AF.Exp)
                    for op in st["ops"]:
                        if op[0] == "mul":
                            _, mi, o = op
                            nc.vector.tensor_mul(
                                ex[:, o:o + 256], ex[:, o:o + 256],
                                mk_sb[:, mi, :])
                        else:
                            _, r0, r1, c0, c1 = op
                            nc.vector.memset(ex[r0:r1, c0:c1].bitcast(FP), 0.0)
                    ct = ctx[st["pair"]]
                    dst = ct[:] if w == 512 else ct[:, st["off"]:st["off"] + 256]
                    nc.tensor.matmul(
                        dst, v_sb[:, h, c, :], ex[:, 0:w],
                        start=st["start"], stop=st["stop"],
                        skip_group_check=True)
                for p in range(NPAIR):
                    rc = rcp.tile([1, 512], FPR, tag="rc", name=f"rc{h}_{p}")
                    # f32r shares the f32 bit layout; only the matmul mode
                    # differs, so the reciprocal itself is full precision
                    with nc.allow_low_precision(reason="f32r == f32 bits"):
                        nc.vector.reciprocal(rc[:], ctx[p][DH:DH + 1, :])
                    bc = pbc.tile([DH, 512], FP, tag="bc", name=f"bc{h}_{p}")
                    nc.tensor.matmul(bc[:], ones_sb[:], rc[:],
                                     start=True, stop=True)
                    # DVE may read only ONE operand from PSUM: stage ctx
                    # through SBUF on ACT, then multiply by the PSUM bcast
                    dst = cT_sb[p0:p0 + 64, mt, p * 512:(p + 1) * 512]
                    nc.scalar.activation(dst, ctx[p][0:DH, :], AF.Copy)
                    nc.vector.tensor_mul(dst, dst, bc[:])

        # ---- phase 3: output projection (partial over this core's features)
        with tc.tile_pool(name="pso", bufs=4, space="PSUM") as po, \
             tc.tile_pool(name="ostg", bufs=4) as ostg:
            for st in range(KB):
                for nt in range(2):
                    ps = po.tile([128, 512], FP, tag="po", name=f"po{st}_{nt}")
                    for kt in range(2):
                        nc.tensor.matmul(
                            ps[:], cT_sb[:, kt, st * 128:(st + 1) * 128],
                            wo_sb[:, kt, nt * 512:(nt + 1) * 512],
                            start=(kt == 0), stop=(kt == 1))
                    ot = ostg.tile([128, 512], FP, tag="ot", name=f"ot{st}_{nt}")
                    nc.scalar.activation(ot[:], ps[:], AF.Copy)
                    nc.sync.dma_start(
                        out[st * 128:(st + 1) * 128, nt * 512:(nt + 1) * 512],
                        ot[:])


_CACHE = {}
TRACE_KWARGS = {}  # test harness may set e.g. dict(tmpdir=...)


def _get_nc():
    if "nc" in _CACHE:
        return _CACHE["nc"], _CACHE["masks"]
    streams, masks = _plan()
    nm = masks.shape[0]
    nc = bacc.Bacc("TRN2", target_bir_lowering=False, debug=False,
                   enable_asserts=False)

    def dp(name, shape, dtype=FPR, is_out=False):
        h = nc.declare_dram_parameter(name, list(shape), dtype, isOutput=is_out)
        return h[:]

    aps = (
        dp("hT", [DM, S]),
        dp("wqT", [DM, F]),
        dp("wkT", [DM, F]),
        dp("wvT", [DM, F]),
        dp("woT", [F, DM]),
        dp("bq2", [2, 128], FP),
        dp("bk2", [2, 128], FP),
        dp("mks", [nm, 128, 256]),
        dp("out", [S, DM], FP, True),
    )
    with tile.TileContext(nc) as tc:
        _emit(tc, streams, nm, aps)
    nc.compile()
    _CACHE["nc"] = nc
    _CACHE["masks"] = masks
    return nc, masks


def make_in_maps(hidden_states, Wq, bq, Wk, bk, Wv, bv, Wo, bo, masks):
    in_maps = []
    f32 = np.float32
    for core in range(NCORES):
        b, fs = core // 4, (core % 4) * F
        in_maps.append({
            "hT": np.ascontiguousarray(hidden_states[b].T, dtype=f32),
            "wqT": np.ascontiguousarray((Wq[fs:fs + F] * SCALE).T, dtype=f32),
            "wkT": np.ascontiguousarray(Wk[fs:fs + F].T, dtype=f32),
            "wvT": np.ascontiguousarray(Wv[fs:fs + F].T, dtype=f32),
            "woT": np.ascontiguousarray(Wo[:, fs:fs + F].T, dtype=f32),
            "bq2": (bq[fs:fs + F] * SCALE).reshape(2, 128).astype(f32),
            "bk2": bk[fs:fs + F].reshape(2, 128).astype(f32),
            "mks": masks,
        })
    return in_maps


def kernel(hidden_states, Wq, bq, Wk, bk, Wv, bv, Wo, bo):
    nc, masks = _get_nc()
    in_maps = make_in_maps(hidden_states, Wq, bq, Wk, bk, Wv, bv, Wo, bo,
                           masks)
    trace = bool(int(os.environ.get("ATTN_TRACE", "0")))
    kw = dict(TRACE_KWARGS) if trace else {}
    res = run_bass_kernel_spmd(nc, in_maps, core_ids=list(range(NCORES)),
                               trace=trace, **kw)
    _CACHE["last_results"] = res
    bias = (bo + Wo @ bv).astype(np.float32)
    out = np.empty((B, S, DM), np.float32)
    for b in range(B):
        acc = res.results[4 * b]["out"].astype(np.float32).copy()
        for c in range(4 * b + 1, 4 * b + 4):
            acc += res.results[c]["out"]
        out[b] = acc + bias
    return out


# revision 16
# speedup vs baseline: 1.0517x; 1.0517x over previous
"""Longformer attention Bass kernel for 8 TRN2 NeuronCores.

Sharding: core c handles batch b = c//4 and heads 4*(c%4) .. 4*(c%4)+3.
Each core computes its 4 heads' attention + the partial output projection;
the host sums the 4 partials per batch element and adds the folded bias.

All matmuls run as float32r (full-rate on TRN2 when N >= 256).
Scores are computed transposed (s^T [keys, q]) so softmax needs no
on-chip transposes: denom comes from a ones-column appended to V, and the
per-query reciprocal is broadcast across partitions with a rank-1 matmul.
"""

import os
import numpy as np

import concourse.bass as bass
import concourse.mybir as mybir
import concourse.tile as tile
from concourse import bacc
from concourse.bass_utils import run_bass_kernel_spmd

# ---- problem constants (hardcoded per contract) ----
B, S, DM = 2, 2048, 1024
H, DH = 16, 64
WINDOW = 128
NG = max(1, int(S * 0.1))  # 204 global tokens
SCALE = 1.0 / np.sqrt(DH)
NCORES = 8
HPC = 4            # heads per core
F = HPC * DH       # 256 per-core head features
KB = S // 128      # 16 key blocks
NPAIR = 4          # ctx psum pair tiles (512 q cols each)

FP = mybir.dt.float32
FPR = mybir.dt.float32r
AF = mybir.ActivationFunctionType


# ---------------------------------------------------------------- planning
def _allow():
    pos = np.arange(S)
    dist = pos[None, :] - pos[:, None]
    window = np.abs(dist) <= WINDOW // 2
    isg = pos < NG
    return window | isg[:, None] | isg[None, :]  # [query i, key j]


def _keyset(g):
    if g == 0:
        return list(range(KB))
    s = {0, 1}
    for c in range(2 * g - 1, 2 * g + 3):
        if 0 <= c < KB:
            s.add(c)
    return sorted(s)


def _plan():
    """Static schedule: list of score/PV streams + deduped mask tiles.

    Each stream: dict(c, q0, w, pair, off, start, stop, ops)
      ops: ('mul', mask_idx, off)            - 256-wide mask multiply
           ('memset', p0, p1, c0, c1)        - zero rows p0:p1 x cols c0:c1
    """
    allowT = _allow().T  # [key, query]
    ksets = [_keyset(g) for g in range(8)]
    last_c = [max(ks) for ks in ksets]
    masks = []
    midx = {}
    streams = []

    def mask_id(sub):
        key = sub.tobytes()
        if key not in midx:
            midx[key] = len(masks)
            masks.append(sub.astype(np.float32))
        return midx[key]

    for c in range(KB):
        if c < 2:
            specs = [(512 * p, 512, p, 0) for p in range(4)]
        else:
            specs = [(256 * g, 256, g // 2, (g % 2) * 256)
                     for g in range(8) if c in ksets[g]]
        for q0, w, pair, off in specs:
            tileT = allowT[c * 128:(c + 1) * 128, q0:q0 + w]
            ops = []
            for o in range(0, w, 256):
                sub = tileT[:, o:o + 256]
                if sub.all():
                    continue
                rfull = sub.all(axis=1)
                rnone = ~sub.any(axis=1)
                cfull = sub.all(axis=0)
                cnone = ~sub.any(axis=0)
                if (rfull | rnone).all() and rnone.any():
                    # rows are all-true or all-false: memset the false rows
                    # (engines require 32-aligned partition offsets)
                    (idx,) = np.nonzero(rnone)
                    p0, p1 = idx.min(), idx.max() + 1
                    if (rnone[p0:p1].all() and p1 - p0 == len(idx)
                            and p0 % 32 == 0 and p1 % 32 == 0):
                        ops.append(("memset", int(p0), int(p1), o, o + 256))
                        continue
                if (cfull | cnone).all() and cnone.any():
                    (idx,) = np.nonzero(cnone)
                    c0, c1 = idx.min(), idx.max() + 1
                    if cnone[c0:c1].all() and c1 - c0 == len(idx):
                        ops.append(("memset", 0, 128, o + int(c0), o + int(c1)))
                        continue
                ops.append(("mul", mask_id(sub), o))
            if w == 512:
                start = (c == 0)
                stop = False  # window streams always finish each half later
            else:
                g = q0 // 256
                start = False
                stop = (c == last_c[g])
            streams.append(dict(c=c, q0=q0, w=w, pair=pair, off=off,
                                start=start, stop=stop, ops=ops))
    return streams, np.stack(masks)  # masks [nm, 128, 256]


# ---------------------------------------------------------------- builder
def _emit(tc, streams, nm, aps):
    nc = tc.nc
    hT, wqT, wkT, wvT, woT, bq2, bk2, mks, out = aps

    with tc.tile_pool(name="const", bufs=1) as const:
        hT_sb = const.tile([128, 8, S], FPR, tag="hT")
        wq_sb = const.tile([128, 8, F], FPR, tag="wq")
        wk_sb = const.tile([128, 8, F], FPR, tag="wk")
        wv_sb = const.tile([128, 8, F], FPR, tag="wv")
        wo_sb = const.tile([128, 2, DM], FPR, tag="wo")
        bq_sb = const.tile([128, 2], FP, tag="bq")
        bk_sb = const.tile([128, 2], FP, tag="bk")
        mk_sb = const.tile([128, nm, 256], FPR, tag="mk")
        ones_sb = const.tile([1, DH], FPR, tag="ones")
        qT_sb = const.tile([128, 2, S], FPR, tag="qT")
        kT_sb = const.tile([128, 2, S], FPR, tag="kT")
        v_sb = const.tile([128, HPC, KB, DH + 1], FPR, tag="v")
        cT_sb = const.tile([128, 2, S], FPR, tag="cT")

        # weights first so the first projection matmuls aren't stuck behind
        # the 8MB hidden-state load; hT then streams in tile by tile
        nc.sync.dma_start(wq_sb[:], wqT.rearrange("(t p) f -> p t f", p=128))
        nc.sync.dma_start(wk_sb[:], wkT.rearrange("(t p) f -> p t f", p=128))
        nc.sync.dma_start(bq_sb[:], bq2.rearrange("t p -> p t"))
        nc.sync.dma_start(bk_sb[:], bk2.rearrange("t p -> p t"))
        hT_r = hT.rearrange("(t p) n -> p t n", p=128)
        for kt in range(8):
            nc.sync.dma_start(hT_sb[:, kt, :], hT_r[:, kt, :])
        nc.sync.dma_start(wv_sb[:], wvT.rearrange("(t p) f -> p t f", p=128))
        nc.sync.dma_start(wo_sb[:], woT.rearrange("(t p) f -> p t f", p=128))
        nc.sync.dma_start(mk_sb[:], mks.rearrange("n p m -> p n m"))
        # Memset can't codegen for f32r APs; the bit pattern of 1.0f is the
        # same, so write through a float32-bitcast view
        nc.vector.memset(ones_sb[:].bitcast(FP), 1.0)
        nc.vector.memset(v_sb[:, :, :, DH:DH + 1].bitcast(FP), 1.0)

        # ---- phase 1: projections q^T, k^T (feat-major) and v (seq-major)
        with tc.tile_pool(name="psproj", bufs=2, space="PSUM") as pp:
            for wsb, bsb, dst in ((wq_sb, bq_sb, qT_sb), (wk_sb, bk_sb, kT_sb)):
                for mt in range(2):
                    for nt in range(4):
                        ps = pp.tile([128, 512], FP, tag="pqk")
                        for kt in range(8):
                            nc.tensor.matmul(
                                ps[:], wsb[:, kt, mt * 128:(mt + 1) * 128],
                                hT_sb[:, kt, nt * 512:(nt + 1) * 512],
                                start=(kt == 0), stop=(kt == 7))
                        nc.vector.tensor_scalar_add(
                            dst[:, mt, nt * 512:(nt + 1) * 512], ps[:],
                            bsb[:, mt:mt + 1])
            for st in range(KB):
                ps = pp.tile([128, F], FP, tag="pv")
                for kt in range(8):
                    nc.tensor.matmul(
                        ps[:], hT_sb[:, kt, st * 128:(st + 1) * 128],
                        wv_sb[:, kt, :], start=(kt == 0), stop=(kt == 7))
                nc.scalar.activation(
                    v_sb[:, :, st, 0:DH],
                    ps.rearrange("p (h d) -> p h d", h=HPC), AF.Copy)

        # ---- phase 2: attention per head
        with tc.tile_pool(name="psT", bufs=3, space="PSUM") as psTp, \
             tc.tile_pool(name="pctx", bufs=1, space="PSUM") as pctx, \
             tc.tile_pool(name="pbc", bufs=1, space="PSUM") as pbc, \
             tc.tile_pool(name="work", bufs=8) as work, \
             tc.tile_pool(name="rcp", bufs=2) as rcp:
            # last stream index touching each ctx pair, so its normalize can
            # be emitted (and scheduled) as soon as the accumulation is done
            pair_done = {}
            for i, st in enumerate(streams):
                pair_done[st["pair"]] = i
            for h in range(HPC):
                p0, mt = 64 * (h % 2), h // 2
                kT_h = kT_sb[p0:p0 + 64, mt, :]
                qT_h = qT_sb[p0:p0 + 64, mt, :]
                ctx = [pctx.tile([DH + 1, 512], FP, tag=f"ctx{p}", bufs=1,
                                 name=f"ctx{p}_{h}")
                       for p in range(NPAIR)]
                for si, st in enumerate(streams):
                    c, q0, w = st["c"], st["q0"], st["w"]
                    ps = psTp.tile([128, 512], FP, tag="sT", name=f"sT{h}_{c}_{q0}")
                    nc.tensor.matmul(
                        ps[:, 0:w], kT_h[:, c * 128:(c + 1) * 128],
                        qT_h[:, q0:q0 + w], start=True, stop=True)
                    ex = work.tile([128, 512], FPR, tag="ex", name=f"ex{h}_{c}_{q0}")
                    nc.scalar.activation(ex[:, 0:w], ps[:, 0:w], AF.Exp)
                    for op in st["ops"]:
                        if op[0] == "mul":
                            _, mi, o = op
                            nc.vector.tensor_mul(
                                ex[:, o:o + 256], ex[:, o:o + 256],
                                mk_sb[:, mi, :])
                        else:
                            _, r0, r1, c0, c1 = op
                            nc.vector.memset(ex[r0:r1, c0:c1].bitcast(FP), 0.0)
                    ct = ctx[st["pair"]]
                    dst = ct[:] if w == 512 else ct[:, st["off"]:st["off"] + 256]
                    nc.tensor.matmul(
                        dst, v_sb[:, h, c, :], ex[:, 0:w],
                        start=st["start"], stop=st["stop"],
                        skip_group_check=True)
                    for p in range(NPAIR):
                        if pair_done[p] != si:
                            continue
                        # normalize this pair as soon as its accumulation is
                        # complete so the chain overlaps remaining streams
                        rc = rcp.tile([1, 512], FPR, tag="rc",
                                      name=f"rc{h}_{p}")
                        # f32r shares the f32 bit layout; only the matmul
                        # mode differs, so the reciprocal is full precision
                        with nc.allow_low_precision(reason="f32r==f32 bits"):
                            nc.vector.reciprocal(rc[:], ctx[p][DH:DH + 1, :])
                        bc = pbc.tile([DH, 512], FP, tag="bc",
                                      name=f"bc{h}_{p}")
                        nc.tensor.matmul(bc[:], ones_sb[:], rc[:],
                                         start=True, stop=True)
                        # DVE may read only ONE operand from PSUM: stage ctx
                        # through SBUF on ACT, then multiply by the PSUM bcast
                        dst = cT_sb[p0:p0 + 64, mt, p * 512:(p + 1) * 512]
                        nc.scalar.activation(dst, ctx[p][0:DH, :], AF.Copy)
                        nc.vector.tensor_mul(dst, dst, bc[:])

        # ---- phase 3: output projection (partial over this core's features)
        with tc.tile_pool(name="pso", bufs=4, space="PSUM") as po, \
             tc.tile_pool(name="ostg", bufs=4) as ostg:
            for st in range(KB):
                for nt in range(2):
                    ps = po.tile([128, 512], FP, tag="po", name=f"po{st}_{nt}")
                    for kt in range(2):
                        nc.tensor.matmul(
                            ps[:], cT_sb[:, kt, st * 128:(st + 1) * 128],
                            wo_sb[:, kt, nt * 512:(nt + 1) * 512],
                            start=(kt == 0), stop=(kt == 1))
                    ot = ostg.tile([128, 512], FP, tag="ot", name=f"ot{st}_{nt}")
                    nc.scalar.activation(ot[:], ps[:], AF.Copy)
                    nc.sync.dma_start(
                        out[st * 128:(st + 1) * 128, nt * 512:(nt + 1) * 512],
                        ot[:])


_CACHE = {}
TRACE_KWARGS = {}  # test harness may set e.g. dict(tmpdir=...)


def _get_nc():
    if "nc" in _CACHE:
        return _CACHE["nc"], _CACHE["masks"]
    streams, masks = _plan()
    nm = masks.shape[0]
    nc = bacc.Bacc("TRN2", target_bir_lowering=False, debug=False,
                   enable_asserts=False)

    def dp(name, shape, dtype=FPR, is_out=False):
        h = nc.declare_dram_parameter(name, list(shape), dtype, isOutput=is_out)
        return h[:]

    aps = (
        dp("hT", [DM, S]),
        dp("wqT", [DM, F]),
        dp("wkT", [DM, F]),
        dp("wvT", [DM, F]),
        dp("woT", [F, DM]),
        dp("bq2", [2, 128], FP),
        dp("bk2", [2, 128], FP),
        dp("mks", [nm, 128, 256]),
        dp("out", [S, DM], FP, True),
    )
    with tile.TileContext(nc) as tc:
        _emit(tc, streams, nm, aps)
    nc.compile()
    _CACHE["nc"] = nc
    _CACHE["masks"] = masks
    return nc, masks


def make_in_maps(hidden_states, Wq, bq, Wk, bk, Wv, bv, Wo, bo, masks):
    in_maps = []
    f32 = np.float32
    for core in range(NCORES):
        b, fs = core // 4, (core % 4) * F
        in_maps.append({
            "hT": np.ascontiguousarray(hidden_states[b].T, dtype=f32),
            "wqT": np.ascontiguousarray((Wq[fs:fs + F] * SCALE).T, dtype=f32),
            "wkT": np.ascontiguousarray(Wk[fs:fs + F].T, dtype=f32),
            "wvT": np.ascontiguousarray(Wv[fs:fs + F].T, dtype=f32),
            "woT": np.ascontiguousarray(Wo[:, fs:fs + F].T, dtype=f32),
            "bq2": (bq[fs:fs + F] * SCALE).reshape(2, 128).astype(f32),
            "bk2": bk[fs:fs + F].reshape(2, 128).astype(f32),
            "mks": masks,
        })
    return in_maps


def kernel(hidden_states, Wq, bq, Wk, bk, Wv, bv, Wo, bo):
    nc, masks = _get_nc()
    in_maps = make_in_maps(hidden_states, Wq, bq, Wk, bk, Wv, bv, Wo, bo,
                           masks)
    trace = bool(int(os.environ.get("ATTN_TRACE", "0")))
    kw = dict(TRACE_KWARGS) if trace else {}
    res = run_bass_kernel_spmd(nc, in_maps, core_ids=list(range(NCORES)),
                               trace=trace, **kw)
    _CACHE["last_results"] = res
    bias = (bo + Wo @ bv).astype(np.float32)
    out = np.empty((B, S, DM), np.float32)
    for b in range(B):
        acc = res.results[4 * b]["out"].astype(np.float32).copy()
        for c in range(4 * b + 1, 4 * b + 4):
            acc += res.results[c]["out"]
        out[b] = acc + bias
    return out
